# revision 14
# baseline (speedup 1.0000x reference)
"""Trainium2 Bass kernel for nn_BiGRU_29901562314941.

Bag-of-sentences BiGRU + word/sentence attention + fc + scatter.

Strategy (8 NeuronCores, data-parallel over bags):
  - 512 bags -> 64 bags/core (= 512 sequences/core, T=64, IN=360).
  - Weights replicated; host pre-transposes everything into the exact
    SBUF images the device wants (feature-on-partitions, batch-on-free),
    in bf16, so the device does zero transposes/conversions.
  - Per core, 2 sub-batches of 256 sequences. Per sub-batch: forward GRU
    sweep, reverse GRU sweep (input projection fused into the recurrence
    PSUM accumulation - xg is never materialized in DRAM), then word
    attention (deferred softmax), accumulating word vectors.
  - Sentence attention + fc on device; host gathers per-core [53,64]
    outputs and scatters rows into the dense (32,8,8,53) result.
"""
import numpy as np
import ml_dtypes

# ---- problem constants (hardcoded per contract) ----
NB, MS, T, IN, H, OUT, DOCS, ENT = 512, 8, 64, 360, 230, 53, 32, 8
NCORES = 8
BAGS_PC = NB // NCORES          # 64 bags/core
B_PC = BAGS_PC * MS             # 512 seqs/core
NSB = 2                         # sub-batches per core
BS = B_PC // NSB                # 256 seqs per sub-batch
HT = 115                        # H = 2*HT
KI = 120                        # IN = 3*KI
H2 = 2 * H                      # 460
G3 = 3 * H                      # 690

BF16 = ml_dtypes.bfloat16

_COMPILED = {}


def _build_program(flags):
    nc = _build_program_nocompile(flags)
    nc.compile()
    return nc


def _build_program_nocompile(flags):
    """Build the per-core Bass/Tile program. flags: dict of
    HAS_BRZ / HAS_BNIH / HAS_BNHH / HAS_BWORD / HAS_BSENT / HAS_BFC."""
    from contextlib import ExitStack
    import concourse.bass as bass
    import concourse.tile as tile
    from concourse import bacc, mybir

    f32 = mybir.dt.float32
    bf16 = mybir.dt.bfloat16
    AF = mybir.ActivationFunctionType
    ALU = mybir.AluOpType
    AX = mybir.AxisListType

    nc = bacc.Bacc("TRN2", target_bir_lowering=False, debug=False,
                   num_devices=NCORES)

    # ---- DRAM I/O ----
    x_img = nc.dram_tensor("x_img", [NSB, T, 3, KI, BS], bf16,
                           kind="ExternalInput").ap()
    wih = nc.dram_tensor("wih", [KI, 2 * 3 * 6 * 128], bf16,
                         kind="ExternalInput").ap()
    whh = nc.dram_tensor("whh", [HT, 2 * 2 * 6 * 128], bf16,
                         kind="ExternalInput").ap()
    wword = nc.dram_tensor("wword", [HT, 16 * 128], bf16,
                           kind="ExternalInput").ap()
    wsent = nc.dram_tensor("wsent", [HT, 16 * 128], bf16,
                           kind="ExternalInput").ap()
    wfc = nc.dram_tensor("wfc", [HT, 4 * OUT], bf16,
                         kind="ExternalInput").ap()
    pword = nc.dram_tensor("pword", [HT, 4], bf16, kind="ExternalInput").ap()
    psent = nc.dram_tensor("psent", [HT, 4], bf16, kind="ExternalInput").ap()
    bias = nc.dram_tensor("bias", [HT, 21], f32, kind="ExternalInput").ap()
    bnhh = nc.dram_tensor("bnhh", [1, 2 * H], bf16, kind="ExternalInput").ap()
    bo = nc.dram_tensor("bo", [OUT, BAGS_PC], f32, kind="ExternalOutput").ap()

    with tile.TileContext(nc) as tc, ExitStack() as ctx:
        wpool = ctx.enter_context(tc.tile_pool(name="weights", bufs=1))
        xpool = ctx.enter_context(tc.tile_pool(name="x", bufs=2))
        gpool = ctx.enter_context(tc.tile_pool(name="gates", bufs=2))
        gpool1 = ctx.enter_context(tc.tile_pool(name="gates1", bufs=1))
        hpool = ctx.enter_context(tc.tile_pool(name="hstore", bufs=1))
        pp = ctx.enter_context(tc.tile_pool(name="ps", bufs=1, space="PSUM"))
        apool = ctx.enter_context(tc.tile_pool(name="attn", bufs=4))
        spool = ctx.enter_context(tc.tile_pool(name="sstage", bufs=2))
        acpool = ctx.enter_context(tc.tile_pool(name="achunk", bufs=2))
        tmppool = ctx.enter_context(tc.tile_pool(name="tmp", bufs=2))
        ptpool = ctx.enter_context(tc.tile_pool(name="partial", bufs=2))
        smpool = ctx.enter_context(tc.tile_pool(name="small", bufs=1))
        wvpool = ctx.enter_context(tc.tile_pool(name="wv", bufs=1))
        dpool = ctx.enter_context(
            tc.tile_pool(name="dram", bufs=1, space="DRAM"))

        # ---- load weights to SBUF ----
        def wtile(name, src, shape, dt):
            t = wpool.tile(shape, dt, tag=name)
            nc.sync.dma_start(t[:], src[:])
            return t

        wih_s = wtile("wih", wih, [KI, 2 * 3 * 6 * 128], bf16)
        whh_s = wtile("whh", whh, [HT, 2 * 2 * 6 * 128], bf16)
        wword_s = wtile("wword", wword, [HT, 16 * 128], bf16)
        wsent_s = wtile("wsent", wsent, [HT, 16 * 128], bf16)
        wfc_s = wtile("wfc", wfc, [HT, 4 * OUT], bf16)
        pword_s = wtile("pword", pword, [HT, 4], bf16)
        psent_s = wtile("psent", psent, [HT, 4], bf16)
        bias_s = wtile("bias", bias, [HT, 21], f32)
        bnhh_s = wtile("bnhh", bnhh, [1, 2 * H], bf16)
        ones_s = wpool.tile([1, BS], bf16, tag="ones")
        nc.vector.memset(ones_s[:], 1.0)

        # weight slice helpers
        def wih_sl(d, kc, m):
            c = ((d * 3 + kc) * 6 + m) * 128
            return wih_s[:, c:c + 128]

        def whh_sl(d, k2, m):
            c = ((d * 2 + k2) * 6 + m) * 128
            return whh_s[:, c:c + 128]

        def wword_sl(kc, m):
            c = (kc * 4 + m) * 128
            return wword_s[:, c:c + 128]

        def wsent_sl(kc, m):
            c = (kc * 4 + m) * 128
            return wsent_s[:, c:c + 128]

        def bias_sl(col):
            return bias_s[:, col:col + 1]

        # persistent word-vector store: [115, (m,b)] cols m*512 + b
        wvbf = wvpool.tile([HT, 4 * B_PC], bf16, tag="wvbf")

        s_dram = dpool.tile([T, BS], f32, tag="s_dram")
        a_dram = dpool.tile([T, BS], bf16, tag="a_dram")

        ATAGS = ["n0", "n1", "xn0", "xn1"]  # 1-bank psum tags reused by attn

        for sb in range(NSB):
            hf = hpool.tile([HT, 2 * T * BS], bf16, tag="hf")
            hr = hpool.tile([HT, 2 * T * BS], bf16, tag="hr")

            # ---------------- GRU sweeps ----------------
            def sweep_step(d, hstore, step, rev):
                trow = (T - 1 - step) if rev else step
                tprev = (trow + 1) if rev else (trow - 1)
                first = step == 0

                xt = xpool.tile([KI, 3 * BS], bf16, tag=f"x{d}")
                nc.sync.dma_start(
                    xt[:],
                    x_img[sb, trow].rearrange("kc f b -> f kc b"))

                def xsl(kc):
                    return xt[:, kc * BS:(kc + 1) * BS]

                def hsl(k2):
                    return hstore[:, k2 * T * BS + tprev * BS:
                                  k2 * T * BS + tprev * BS + BS]

                # xn psum: W_ih_n @ x_t (consumed straight from PSUM)
                xnp = pp.tile([128, 2 * BS], f32, tag=f"xn{d}")
                for m in (0, 1):
                    o = xnp[:, m * BS:(m + 1) * BS]
                    for kc in range(3):
                        nc.tensor.matmul(o, wih_sl(d, kc, 4 + m), xsl(kc),
                                         start=(kc == 0), stop=(kc == 2))
                xnv = xnp[0:HT, :]

                # r,z psum: W_ih_rz @ x + W_hh_rz @ h
                rzp = pp.tile([128, 4 * BS], f32, tag=f"rz{d}")
                for m in range(4):
                    o = rzp[:, m * BS:(m + 1) * BS]
                    for kc in range(3):
                        nc.tensor.matmul(o, wih_sl(d, kc, m), xsl(kc),
                                         start=(kc == 0),
                                         stop=(kc == 2 and first))
                    if not first:
                        for k2 in (0, 1):
                            nc.tensor.matmul(o, whh_sl(d, k2, m), hsl(k2),
                                             start=False, stop=(k2 == 1))
                rzs = gpool.tile([HT, 4 * BS], bf16, tag=f"rz{d}")
                if flags["HAS_BRZ"]:
                    for m in range(4):
                        nc.scalar.activation(
                            rzs[:, m * BS:(m + 1) * BS],
                            rzp[0:HT, m * BS:(m + 1) * BS],
                            AF.Sigmoid, bias=bias_sl(d * 4 + m))
                else:
                    nc.scalar.activation(rzs[:], rzp[0:HT, :], AF.Sigmoid)

                # n-gate hidden part
                need_np = (not first) or flags["HAS_BNHH"]
                if need_np:
                    np_ = pp.tile([128, 2 * BS], f32, tag=f"n{d}")
                    for m in (0, 1):
                        o = np_[:, m * BS:(m + 1) * BS]
                        started = False
                        if flags["HAS_BNHH"]:
                            nc.tensor.matmul(
                                o[0:HT, :],
                                bnhh_s[:, d * H + m * HT:
                                       d * H + (m + 1) * HT],
                                ones_s[:], start=True, stop=first)
                            started = True
                        if not first:
                            for k2 in (0, 1):
                                nc.tensor.matmul(o, whh_sl(d, k2, 4 + m),
                                                 hsl(k2),
                                                 start=(not started and
                                                        k2 == 0),
                                                 stop=(k2 == 1))
                    tt = gpool1.tile([HT, 2 * BS], bf16, tag=f"t{d}")
                    nc.vector.tensor_tensor(tt[:], rzs[:, 0:2 * BS],
                                            np_[0:HT, :], ALU.mult)
                    nc.vector.tensor_tensor(tt[:], tt[:], xnv, ALU.add)
                    tanh_in = tt[:]
                else:
                    tanh_in = xnv
                ns = gpool1.tile([HT, 2 * BS], bf16, tag=f"n{d}")
                if flags["HAS_BNIH"]:
                    for m in (0, 1):
                        nc.scalar.activation(
                            ns[:, m * BS:(m + 1) * BS],
                            tanh_in[:, m * BS:(m + 1) * BS] if need_np
                            else xnp[0:HT, m * BS:(m + 1) * BS],
                            AF.Tanh, bias=bias_sl(8 + d * 2 + m))
                else:
                    nc.scalar.activation(ns[:], tanh_in, AF.Tanh)

                # h2 = n + z*(h_prev - n), written into hstore column t
                h3 = hstore[:].rearrange("p (m tb) -> p m tb", m=2)
                hview = h3[:, :, trow * BS:(trow + 1) * BS]
                ns3 = ns[:].rearrange("p (m b) -> p m b", m=2)
                z3 = rzs[:, 2 * BS:4 * BS].rearrange("p (m b) -> p m b", m=2)
                dd = gpool1.tile([HT, 2 * BS], bf16, tag=f"d{d}")
                dd3 = dd[:].rearrange("p (m b) -> p m b", m=2)
                if first:
                    nc.vector.tensor_scalar_mul(dd[:], ns[:], -1.0)
                else:
                    hprev3 = h3[:, :, tprev * BS:(tprev + 1) * BS]
                    nc.vector.tensor_tensor(dd3, hprev3, ns3, ALU.subtract)
                nc.vector.tensor_tensor(dd3, z3, dd3, ALU.mult)
                nc.vector.tensor_tensor(hview, ns3, dd3, ALU.add)

            for step in range(T):
                sweep_step(0, hf, step, rev=False)
                sweep_step(1, hr, step, rev=True)

            # ---------------- word attention ----------------
            def outchunk(kc, lo, n):
                src = hf if kc < 2 else hr
                c = (kc % 2) * T * BS + lo
                return src[:, c:c + n]

            NW = (T * BS) // 512  # windows of 512 over (t,b)
            for w in range(NW):
                us_tiles = []
                for m in range(4):
                    up = pp.tile([128, 512], f32, tag=ATAGS[m])
                    for kc in range(4):
                        nc.tensor.matmul(up[:], wword_sl(kc, m),
                                         outchunk(kc, w * 512, 512),
                                         start=(kc == 0), stop=(kc == 3))
                    us = apool.tile([HT, 512], bf16, tag="us")
                    if flags["HAS_BWORD"]:
                        nc.scalar.activation(us[:], up[0:HT, :], AF.Tanh,
                                             bias=bias_sl(12 + m))
                    else:
                        nc.scalar.activation(us[:], up[0:HT, :], AF.Tanh)
                    us_tiles.append(us)
                sp = pp.tile([1, 512], f32, tag="rz0")
                for m in range(4):
                    nc.tensor.matmul(sp[:], pword_s[:, m:m + 1],
                                     us_tiles[m][:],
                                     start=(m == 0), stop=(m == 3))
                sst = spool.tile([1, 512], f32, tag="sst")
                nc.scalar.copy(sst[:], sp[:])
                tlo = (w * 512) // BS
                nc.sync.dma_start(s_dram[tlo:tlo + (512 // BS), :], sst[:])

            # softmax over T: flip scores to [b, t] layout via DRAM bounce
            for k in range(BS // 128):
                sT = smpool.tile([128, T], f32, tag="sT")
                nc.sync.dma_start(
                    sT[:],
                    s_dram[:, k * 128:(k + 1) * 128].rearrange("t b -> b t"))
                nmx = smpool.tile([128, 1], f32, tag="nmx")
                nc.vector.tensor_reduce(nmx[:], sT[:], AX.X, ALU.max,
                                        negate=True)
                ee = smpool.tile([128, T], f32, tag="ee")
                nc.scalar.activation(ee[:], sT[:], AF.Exp, bias=nmx[:])
                sm = smpool.tile([128, 1], f32, tag="sm")
                nc.vector.tensor_reduce(sm[:], ee[:], AX.X, ALU.add)
                inv = smpool.tile([128, 1], f32, tag="inv")
                nc.vector.reciprocal(inv[:], sm[:])
                abf = smpool.tile([128, T], bf16, tag="abf")
                nc.vector.tensor_scalar(abf[:], ee[:], inv[:], None,
                                        op0=ALU.mult)
                nc.sync.dma_start(
                    a_dram[:, k * 128:(k + 1) * 128].rearrange("t b -> b t"),
                    abf[:])

            # word_vec[f,b] = sum_t alpha[t,b] * out[f,(t,b)]
            TC = 4  # t-chunk
            for tc8 in range(T // TC):
                ach = acpool.tile([HT, TC * BS], bf16, tag="ach")
                nc.sync.dma_start(
                    ach[:],
                    a_dram[tc8 * TC:(tc8 + 1) * TC, :].partition_broadcast(HT))
                for m in range(4):
                    tmp = tmppool.tile([HT, TC * BS], bf16, tag="tmp")
                    nc.vector.tensor_tensor(
                        tmp[:], outchunk(m, tc8 * TC * BS, TC * BS), ach[:],
                        ALU.mult)
                    pt = ptpool.tile([HT, BS], f32, tag="pt")
                    nc.vector.tensor_reduce(
                        pt[:], tmp[:].rearrange("p (t b) -> p b t", t=TC),
                        AX.X, ALU.add)
                    wvsl = wvbf[:, m * B_PC + sb * BS:
                                m * B_PC + sb * BS + BS]
                    if tc8 == 0:
                        nc.vector.tensor_copy(wvsl, pt[:])
                    else:
                        nc.vector.tensor_tensor(wvsl, wvsl, pt[:], ALU.add)

        # ---------------- sentence attention + fc ----------------
        u2_tiles = []
        for m in range(4):
            u2p = pp.tile([128, B_PC], f32, tag=ATAGS[m])
            for kc in range(4):
                nc.tensor.matmul(u2p[:], wsent_sl(kc, m),
                                 wvbf[:, kc * B_PC:(kc + 1) * B_PC],
                                 start=(kc == 0), stop=(kc == 3))
            u2s = apool.tile([HT, B_PC], bf16, tag="us")
            if flags["HAS_BSENT"]:
                nc.scalar.activation(u2s[:], u2p[0:HT, :], AF.Tanh,
                                     bias=bias_sl(16 + m))
            else:
                nc.scalar.activation(u2s[:], u2p[0:HT, :], AF.Tanh)
            u2_tiles.append(u2s)
        s2p = pp.tile([1, B_PC], f32, tag="rz0")
        for m in range(4):
            nc.tensor.matmul(s2p[:], psent_s[:, m:m + 1], u2_tiles[m][:],
                             start=(m == 0), stop=(m == 3))
        e2 = smpool.tile([1, B_PC], f32, tag="e2")
        nc.scalar.activation(e2[:], s2p[:], AF.Exp)
        sm2 = smpool.tile([1, BAGS_PC], f32, tag="sm2")
        nc.vector.tensor_reduce(
            sm2[:], e2[:].rearrange("p (g s) -> p g s", s=MS), AX.X, ALU.add)
        inv2 = smpool.tile([1, BAGS_PC], f32, tag="inv2")
        nc.vector.reciprocal(inv2[:], sm2[:])
        # broadcast e2/inv2 across partitions via DRAM bounce
        e2_d = dpool.tile([1, B_PC], f32, tag="e2_d")
        inv2_d = dpool.tile([1, BAGS_PC], f32, tag="inv2_d")
        nc.sync.dma_start(e2_d[:], e2[:])
        nc.sync.dma_start(inv2_d[:], inv2[:])
        e2r = acpool.tile([HT, B_PC], f32, tag="ach")
        nc.sync.dma_start(e2r[:], e2_d[0].partition_broadcast(HT))
        inv2r = ptpool.tile([HT, BAGS_PC], f32, tag="pt")
        nc.sync.dma_start(inv2r[:], inv2_d[0].partition_broadcast(HT))

        sv_tiles = []
        for m in range(4):
            tmp2 = tmppool.tile([HT, B_PC], bf16, tag="tmp")
            nc.vector.tensor_tensor(
                tmp2[:], wvbf[:, m * B_PC:(m + 1) * B_PC], e2r[:], ALU.mult)
            sv = smpool.tile([HT, BAGS_PC], f32, tag=f"sv{m}")
            nc.vector.tensor_reduce(
                sv[:], tmp2[:].rearrange("p (g s) -> p g s", s=MS),
                AX.X, ALU.add)
            svb = smpool.tile([HT, BAGS_PC], bf16, tag=f"svb{m}")
            nc.vector.tensor_tensor(svb[:], sv[:], inv2r[:], ALU.mult)
            sv_tiles.append(svb)
        fcp = pp.tile([OUT, BAGS_PC], f32, tag="rz1")
        for m in range(4):
            nc.tensor.matmul(fcp[:], wfc_s[:, m * OUT:(m + 1) * OUT],
                             sv_tiles[m][:], start=(m == 0), stop=(m == 3))
        bos = smpool.tile([OUT, BAGS_PC], f32, tag="bos")
        if flags["HAS_BFC"]:
            nc.scalar.activation(bos[:], fcp[:], AF.Identity,
                                 bias=bias_s[0:OUT, 20:21])
        else:
            nc.scalar.copy(bos[:], fcp[:])
        nc.sync.dma_start(bo[:], bos[:])

    return nc


def _host_prep(inputs):
    """Build per-core input maps (weight images shared, x sharded)."""
    def as_np(a, dt=np.float32):
        return np.asarray(a, dtype=dt)

    bag = as_np(inputs["bag"])          # [512, 8, 64, 360]
    W_ih = [as_np(inputs["W_ih_f"]), as_np(inputs["W_ih_r"])]   # [690,360]
    W_hh = [as_np(inputs["W_hh_f"]), as_np(inputs["W_hh_r"])]   # [690,230]
    b_ih = [as_np(inputs["b_ih_f"]), as_np(inputs["b_ih_r"])]
    b_hh = [as_np(inputs["b_hh_f"]), as_np(inputs["b_hh_r"])]
    W_word = as_np(inputs["W_word"])    # [460,460]
    b_word = as_np(inputs["b_word"])
    proj_word = as_np(inputs["proj_word"])
    W_sent = as_np(inputs["W_sent"])
    b_sent = as_np(inputs["b_sent"])
    proj_sent = as_np(inputs["proj_sent"])
    fc_W = as_np(inputs["fc_W"])        # [53,460]
    fc_b = as_np(inputs["fc_b"])

    # weight images
    def padm(a):
        # pad last (M) dim from HT to 128 so FWL (fast weight load) engages
        pad = np.zeros(a.shape[:-1] + (128 - HT,), a.dtype)
        return np.concatenate([a, pad], axis=-1)

    wih = np.stack(
        [W.reshape(6, HT, 3, KI).transpose(3, 2, 0, 1) for W in W_ih],
        axis=1)  # [KI, 2, 3, 6, HT]
    wih = padm(np.ascontiguousarray(wih)).reshape(KI, 2 * 3 * 6 * 128)
    whh = np.stack(
        [W.reshape(6, HT, 2, HT).transpose(3, 2, 0, 1) for W in W_hh],
        axis=1)  # [HT, 2, 2, 6, HT]
    whh = padm(np.ascontiguousarray(whh)).reshape(HT, 2 * 2 * 6 * 128)
    wword = padm(np.ascontiguousarray(
        W_word.reshape(4, HT, 4, HT).transpose(1, 0, 2, 3))).reshape(HT, -1)
    wsent = padm(np.ascontiguousarray(
        W_sent.reshape(4, HT, 4, HT).transpose(1, 0, 2, 3))).reshape(HT, -1)
    wfc = np.ascontiguousarray(
        fc_W.T.reshape(4, HT, OUT).transpose(1, 0, 2)).reshape(HT, -1)
    pword = np.ascontiguousarray(proj_word.reshape(4, HT).T)
    psent = np.ascontiguousarray(proj_sent.reshape(4, HT).T)

    bias = np.zeros((HT, 21), np.float32)
    for d in range(2):
        brz = (b_ih[d][:H2] + b_hh[d][:H2]).reshape(4, HT)
        bias[:, d * 4:(d + 1) * 4] = brz.T
        bias[:, 8 + d * 2:8 + (d + 1) * 2] = \
            b_ih[d][H2:].reshape(2, HT).T
    bias[:, 12:16] = b_word.reshape(4, HT).T
    bias[:, 16:20] = b_sent.reshape(4, HT).T
    bias[:OUT, 20] = fc_b
    bnhh = np.concatenate([b_hh[0][H2:], b_hh[1][H2:]])[None, :]

    flags = {
        "HAS_BRZ": bool(np.any(bias[:, 0:8])),
        "HAS_BNIH": bool(np.any(bias[:, 8:12])),
        "HAS_BNHH": bool(np.any(bnhh)),
        "HAS_BWORD": bool(np.any(bias[:, 12:16])),
        "HAS_BSENT": bool(np.any(bias[:, 16:20])),
        "HAS_BFC": bool(np.any(fc_b)),
    }

    shared = {
        "wih": wih.astype(BF16), "whh": whh.astype(BF16),
        "wword": wword.astype(BF16), "wsent": wsent.astype(BF16),
        "wfc": wfc.astype(BF16), "pword": pword.astype(BF16),
        "psent": psent.astype(BF16), "bias": bias,
        "bnhh": bnhh.astype(BF16),
    }

    in_maps = []
    for core in range(NCORES):
        bc = bag[core * BAGS_PC:(core + 1) * BAGS_PC]   # [64,8,64,360]
        x = bc.reshape(NSB, BS, T, IN).transpose(0, 2, 3, 1)  # [sb,t,f,b]
        x = np.ascontiguousarray(x).reshape(NSB, T, 3, KI, BS)
        m = dict(shared)
        m["x_img"] = x.astype(BF16)
        in_maps.append(m)
    return in_maps, flags


def kernel(**inputs):
    in_maps, flags = _host_prep(inputs)

    key = tuple(sorted(flags.items()))
    if key not in _COMPILED:
        _COMPILED[key] = _build_program(flags)
    nc = _COMPILED[key]

    from concourse.bass_utils import run_bass_kernel_spmd
    res = run_bass_kernel_spmd(nc, in_maps, core_ids=list(range(NCORES)))

    pairs = np.asarray(inputs["pairs"], dtype=np.int64)  # [512,3]
    bag_out = np.concatenate(
        [res.results[c]["bo"].astype(np.float32).T for c in range(NCORES)],
        axis=0)  # [512, 53]
    out = np.zeros((DOCS, ENT, ENT, OUT), np.float32)
    out[pairs[:, 0], pairs[:, 1], pairs[:, 2]] = bag_out
    return out


# revision 18
# speedup vs baseline: 1.0529x; 1.0529x over previous
"""Trainium2 Bass kernel for nn_BiGRU_29901562314941.

Bag-of-sentences BiGRU + word/sentence attention + fc + scatter.

Strategy (8 NeuronCores, data-parallel over bags):
  - 512 bags -> 64 bags/core (= 512 sequences/core, T=64, IN=360).
  - Weights replicated; host pre-transposes everything into the exact
    SBUF images the device wants (feature-on-partitions, batch-on-free),
    in bf16, so the device does zero transposes/conversions.
  - Per core, 2 sub-batches of 256 sequences. Per sub-batch: forward GRU
    sweep, reverse GRU sweep (input projection fused into the recurrence
    PSUM accumulation - xg is never materialized in DRAM), then word
    attention (deferred softmax), accumulating word vectors.
  - Sentence attention + fc on device; host gathers per-core [53,64]
    outputs and scatters rows into the dense (32,8,8,53) result.
"""
import numpy as np
import ml_dtypes

# ---- problem constants (hardcoded per contract) ----
NB, MS, T, IN, H, OUT, DOCS, ENT = 512, 8, 64, 360, 230, 53, 32, 8
NCORES = 8
BAGS_PC = NB // NCORES          # 64 bags/core
B_PC = BAGS_PC * MS             # 512 seqs/core
NSB = 2                         # sub-batches per core
BS = B_PC // NSB                # 256 seqs per sub-batch
HT = 115                        # H = 2*HT
KI = 120                        # IN = 3*KI
H2 = 2 * H                      # 460
G3 = 3 * H                      # 690

BF16 = ml_dtypes.bfloat16

_COMPILED = {}


def _build_program(flags):
    nc = _build_program_nocompile(flags)
    nc.compile()
    return nc


def _build_program_nocompile(flags):
    """Build the per-core Bass/Tile program. flags: dict of
    HAS_BRZ / HAS_BNIH / HAS_BNHH / HAS_BWORD / HAS_BSENT / HAS_BFC."""
    from contextlib import ExitStack
    import concourse.bass as bass
    import concourse.tile as tile
    from concourse import bacc, mybir

    f32 = mybir.dt.float32
    bf16 = mybir.dt.bfloat16
    AF = mybir.ActivationFunctionType
    ALU = mybir.AluOpType
    AX = mybir.AxisListType

    nc = bacc.Bacc("TRN2", target_bir_lowering=False, debug=False,
                   num_devices=NCORES)

    # ---- DRAM I/O ----
    x_img = nc.dram_tensor("x_img", [NSB, T, 3, 128, BS], bf16,
                           kind="ExternalInput").ap()
    wih = nc.dram_tensor("wih", [128, 2 * 3 * 6 * 128], bf16,
                         kind="ExternalInput").ap()
    whh = nc.dram_tensor("whh", [128, 2 * 2 * 6 * 128], bf16,
                         kind="ExternalInput").ap()
    wword = nc.dram_tensor("wword", [128, 16 * 128], bf16,
                           kind="ExternalInput").ap()
    wsent = nc.dram_tensor("wsent", [128, 16 * 128], bf16,
                           kind="ExternalInput").ap()
    wfc = nc.dram_tensor("wfc", [HT, 4 * OUT], bf16,
                         kind="ExternalInput").ap()
    pword = nc.dram_tensor("pword", [HT, 4], bf16, kind="ExternalInput").ap()
    psent = nc.dram_tensor("psent", [HT, 4], bf16, kind="ExternalInput").ap()
    bias = nc.dram_tensor("bias", [HT, 21], f32, kind="ExternalInput").ap()
    bnhh = nc.dram_tensor("bnhh", [1, 2 * H], bf16, kind="ExternalInput").ap()
    bo = nc.dram_tensor("bo", [OUT, BAGS_PC], f32, kind="ExternalOutput").ap()

    with tile.TileContext(nc) as tc, ExitStack() as ctx:
        wpool = ctx.enter_context(tc.tile_pool(name="weights", bufs=1))
        xpool = ctx.enter_context(tc.tile_pool(name="x", bufs=2))
        xnpool = ctx.enter_context(tc.tile_pool(name="xn", bufs=2))
        gpool = ctx.enter_context(tc.tile_pool(name="gates", bufs=2))
        gpool1 = ctx.enter_context(tc.tile_pool(name="gates1", bufs=1))
        hpool = ctx.enter_context(tc.tile_pool(name="hstore", bufs=1))
        pp = ctx.enter_context(tc.tile_pool(name="ps", bufs=1, space="PSUM"))
        apool = ctx.enter_context(tc.tile_pool(name="attn", bufs=4))
        spool = ctx.enter_context(tc.tile_pool(name="sstage", bufs=2))
        acpool = ctx.enter_context(tc.tile_pool(name="achunk", bufs=2))
        tmppool = ctx.enter_context(tc.tile_pool(name="tmp", bufs=2))
        ptpool = ctx.enter_context(tc.tile_pool(name="partial", bufs=2))
        smpool = ctx.enter_context(tc.tile_pool(name="small", bufs=1))
        wvpool = ctx.enter_context(tc.tile_pool(name="wv", bufs=1))
        dpool = ctx.enter_context(
            tc.tile_pool(name="dram", bufs=1, space="DRAM"))

        # ---- load weights to SBUF ----
        def wtile(name, src, shape, dt):
            t = wpool.tile(shape, dt, tag=name)
            nc.sync.dma_start(t[:], src[:])
            return t

        wih_s = wtile("wih", wih, [128, 2 * 3 * 6 * 128], bf16)
        whh_s = wtile("whh", whh, [128, 2 * 2 * 6 * 128], bf16)
        wword_s = wtile("wword", wword, [128, 16 * 128], bf16)
        wsent_s = wtile("wsent", wsent, [128, 16 * 128], bf16)
        wfc_s = wtile("wfc", wfc, [HT, 4 * OUT], bf16)
        pword_s = wtile("pword", pword, [HT, 4], bf16)
        psent_s = wtile("psent", psent, [HT, 4], bf16)
        bias_s = wtile("bias", bias, [HT, 21], f32)
        bnhh_s = wtile("bnhh", bnhh, [1, 2 * H], bf16)
        ones_s = wpool.tile([1, BS], bf16, tag="ones")
        nc.vector.memset(ones_s[:], 1.0)

        # weight slice helpers
        def wih_sl(d, kc, m):
            c = ((d * 3 + kc) * 6 + m) * 128
            return wih_s[:, c:c + 128]

        def whh_sl(d, k2, m):
            c = ((d * 2 + k2) * 6 + m) * 128
            return whh_s[:, c:c + 128]

        def wword_sl(kc, m):
            c = (kc * 4 + m) * 128
            return wword_s[:, c:c + 128]

        def wsent_sl(kc, m):
            c = (kc * 4 + m) * 128
            return wsent_s[:, c:c + 128]

        def bias_sl(col):
            return bias_s[:, col:col + 1]

        # persistent word-vector store: [115(+pad), (m,b)] cols m*512 + b
        wvbf = wvpool.tile([128, 4 * B_PC], bf16, tag="wvbf")
        nc.gpsimd.memset(wvbf[96:128, :], 0.0)

        s_dram = dpool.tile([T, BS], f32, tag="s_dram")
        a_dram = dpool.tile([T, BS], bf16, tag="a_dram")

        for sb in range(NSB):
            hf = hpool.tile([128, 2 * T * BS], bf16, tag="hf")
            hr = hpool.tile([128, 2 * T * BS], bf16, tag="hr")
            # zero the K-padding partitions once (matmul rhs reads [0:128])
            nc.gpsimd.memset(hf[96:128, :], 0.0)
            nc.gpsimd.memset(hr[96:128, :], 0.0)

            # ---------------- GRU sweeps ----------------
            def sweep_step(d, hstore, step, rev):
                trow = (T - 1 - step) if rev else step
                tprev = (trow + 1) if rev else (trow - 1)
                first = step == 0

                xt = xpool.tile([128, 3 * BS], bf16, tag=f"x{d}")
                nc.sync.dma_start(
                    xt[:],
                    x_img[sb, trow].rearrange("kc f b -> f kc b"))

                def xsl(kc):
                    return xt[:, kc * BS:(kc + 1) * BS]

                def hsl(k2):
                    return hstore[:, k2 * T * BS + tprev * BS:
                                  k2 * T * BS + tprev * BS + BS]

                # xn psum: W_ih_n @ x_t -> SBUF bf16 (prefetchable)
                xnp = pp.tile([128, 2 * BS], f32, tag="xn")
                for m in (0, 1):
                    o = xnp[:, m * BS:(m + 1) * BS]
                    for kc in range(3):
                        nc.tensor.matmul(o, wih_sl(d, kc, 4 + m), xsl(kc),
                                         start=(kc == 0), stop=(kc == 2))
                xns = xnpool.tile([HT, 2 * BS], bf16, tag=f"xn{d}")
                if flags["HAS_BNIH"]:
                    for m in (0, 1):
                        nc.scalar.activation(
                            xns[:, m * BS:(m + 1) * BS],
                            xnp[0:HT, m * BS:(m + 1) * BS],
                            AF.Identity, bias=bias_sl(8 + d * 2 + m))
                else:
                    nc.scalar.copy(xns[:], xnp[0:HT, :])

                # r,z psum: W_ih_rz @ x + W_hh_rz @ h
                rzp = pp.tile([128, 4 * BS], f32, tag=f"rz{d}")
                for m in range(4):
                    o = rzp[:, m * BS:(m + 1) * BS]
                    for kc in range(3):
                        nc.tensor.matmul(o, wih_sl(d, kc, m), xsl(kc),
                                         start=(kc == 0),
                                         stop=(kc == 2 and first))
                    if not first:
                        for k2 in (0, 1):
                            nc.tensor.matmul(o, whh_sl(d, k2, m), hsl(k2),
                                             start=False, stop=(k2 == 1))
                rzs = gpool.tile([HT, 4 * BS], bf16, tag=f"rz{d}")
                if flags["HAS_BRZ"]:
                    for m in range(4):
                        nc.scalar.activation(
                            rzs[:, m * BS:(m + 1) * BS],
                            rzp[0:HT, m * BS:(m + 1) * BS],
                            AF.Sigmoid, bias=bias_sl(d * 4 + m))
                else:
                    nc.scalar.activation(rzs[:], rzp[0:HT, :], AF.Sigmoid)

                # n-gate hidden part
                need_np = (not first) or flags["HAS_BNHH"]
                if need_np:
                    np_ = pp.tile([128, 2 * BS], f32, tag="n")
                    for m in (0, 1):
                        o = np_[:, m * BS:(m + 1) * BS]
                        started = False
                        if flags["HAS_BNHH"]:
                            nc.tensor.matmul(
                                o[0:HT, :],
                                bnhh_s[:, d * H + m * HT:
                                       d * H + (m + 1) * HT],
                                ones_s[:], start=True, stop=first)
                            started = True
                        if not first:
                            for k2 in (0, 1):
                                nc.tensor.matmul(o, whh_sl(d, k2, 4 + m),
                                                 hsl(k2),
                                                 start=(not started and
                                                        k2 == 0),
                                                 stop=(k2 == 1))
                    tt = gpool1.tile([HT, 2 * BS], bf16, tag=f"t{d}")
                    nc.vector.tensor_tensor(tt[:], rzs[:, 0:2 * BS],
                                            np_[0:HT, :], ALU.mult)
                    nc.vector.tensor_tensor(tt[:], tt[:], xns[:], ALU.add)
                    tanh_in = tt[:]
                else:
                    tanh_in = xns[:]
                ns = gpool1.tile([HT, 2 * BS], bf16, tag=f"n{d}")
                nc.scalar.activation(ns[:], tanh_in, AF.Tanh)

                # h2 = n + z*(h_prev - n), written into hstore column t
                h3 = hstore[0:HT, :].rearrange("p (m tb) -> p m tb", m=2)
                hview = h3[:, :, trow * BS:(trow + 1) * BS]
                ns3 = ns[:].rearrange("p (m b) -> p m b", m=2)
                z3 = rzs[:, 2 * BS:4 * BS].rearrange("p (m b) -> p m b", m=2)
                dd = gpool1.tile([HT, 2 * BS], bf16, tag=f"d{d}")
                dd3 = dd[:].rearrange("p (m b) -> p m b", m=2)
                if first:
                    nc.vector.tensor_scalar_mul(dd[:], ns[:], -1.0)
                else:
                    hprev3 = h3[:, :, tprev * BS:(tprev + 1) * BS]
                    nc.vector.tensor_tensor(dd3, hprev3, ns3, ALU.subtract)
                nc.vector.tensor_tensor(dd3, z3, dd3, ALU.mult)
                nc.vector.tensor_tensor(hview, ns3, dd3, ALU.add)

            # ---- word attention for one completed row t ----
            def outchunk(kc, lo, n, p=128):
                h_src = hf if kc < 2 else hr
                c = (kc % 2) * T * BS + lo
                return h_src[0:p, c:c + n]

            def attn_row(t):
                us_tiles = []
                for m in range(4):
                    up = pp.tile([128, BS], f32, tag="u")
                    for kc in range(4):
                        nc.tensor.matmul(up[:], wword_sl(kc, m),
                                         outchunk(kc, t * BS, BS),
                                         start=(kc == 0), stop=(kc == 3))
                    us = apool.tile([HT, BS], bf16, tag="us")
                    if flags["HAS_BWORD"]:
                        nc.scalar.activation(us[:], up[0:HT, :], AF.Tanh,
                                             bias=bias_sl(12 + m))
                    else:
                        nc.scalar.activation(us[:], up[0:HT, :], AF.Tanh)
                    us_tiles.append(us)
                sp = pp.tile([1, BS], f32, tag="u")
                for m in range(4):
                    nc.tensor.matmul(sp[:], pword_s[:, m:m + 1],
                                     us_tiles[m][:],
                                     start=(m == 0), stop=(m == 3))
                sst = spool.tile([1, BS], f32, tag="sst")
                nc.scalar.copy(sst[:], sp[:])
                nc.sync.dma_start(s_dram[t:t + 1, :], sst[:])

            for step in range(T):
                sweep_step(0, hf, step, rev=False)
                sweep_step(1, hr, step, rev=True)
                if step >= T // 2:
                    attn_row(step)
                    attn_row(T - 1 - step)

            # softmax over T: flip scores to [b, t] layout via DRAM bounce
            for k in range(BS // 128):
                sT = smpool.tile([128, T], f32, tag="sT")
                nc.sync.dma_start(
                    sT[:],
                    s_dram[:, k * 128:(k + 1) * 128].rearrange("t b -> b t"))
                nmx = smpool.tile([128, 1], f32, tag="nmx")
                nc.vector.tensor_reduce(nmx[:], sT[:], AX.X, ALU.max,
                                        negate=True)
                ee = smpool.tile([128, T], f32, tag="ee")
                nc.scalar.activation(ee[:], sT[:], AF.Exp, bias=nmx[:])
                sm = smpool.tile([128, 1], f32, tag="sm")
                nc.vector.tensor_reduce(sm[:], ee[:], AX.X, ALU.add)
                inv = smpool.tile([128, 1], f32, tag="inv")
                nc.vector.reciprocal(inv[:], sm[:])
                abf = smpool.tile([128, T], bf16, tag="abf")
                nc.vector.tensor_scalar(abf[:], ee[:], inv[:], None,
                                        op0=ALU.mult)
                nc.sync.dma_start(
                    a_dram[:, k * 128:(k + 1) * 128].rearrange("t b -> b t"),
                    abf[:])

            # word_vec[f,b] = sum_t alpha[t,b] * out[f,(t,b)]
            TC = 4  # t-chunk
            for tc8 in range(T // TC):
                ach = acpool.tile([HT, TC * BS], bf16, tag="ach")
                nc.sync.dma_start(
                    ach[:],
                    a_dram[tc8 * TC:(tc8 + 1) * TC, :].partition_broadcast(HT))
                for m in range(4):
                    tmp = tmppool.tile([HT, TC * BS], bf16, tag="tmp")
                    nc.vector.tensor_tensor(
                        tmp[:], outchunk(m, tc8 * TC * BS, TC * BS, HT),
                        ach[:], ALU.mult)
                    pt = ptpool.tile([HT, BS], f32, tag="pt")
                    nc.vector.tensor_reduce(
                        pt[:], tmp[:].rearrange("p (t b) -> p b t", t=TC),
                        AX.X, ALU.add)
                    wvsl = wvbf[0:HT, m * B_PC + sb * BS:
                                m * B_PC + sb * BS + BS]
                    if tc8 == 0:
                        nc.vector.tensor_copy(wvsl, pt[:])
                    else:
                        nc.vector.tensor_tensor(wvsl, wvsl, pt[:], ALU.add)

        # ---------------- sentence attention + fc ----------------
        u2_tiles = []
        for m in range(4):
            u2p = pp.tile([128, B_PC], f32, tag="u")
            for kc in range(4):
                nc.tensor.matmul(u2p[:], wsent_sl(kc, m),
                                 wvbf[:, kc * B_PC:(kc + 1) * B_PC],
                                 start=(kc == 0), stop=(kc == 3))
            u2s = apool.tile([HT, B_PC], bf16, tag="us")
            if flags["HAS_BSENT"]:
                nc.scalar.activation(u2s[:], u2p[0:HT, :], AF.Tanh,
                                     bias=bias_sl(16 + m))
            else:
                nc.scalar.activation(u2s[:], u2p[0:HT, :], AF.Tanh)
            u2_tiles.append(u2s)
        s2p = pp.tile([1, B_PC], f32, tag="u")
        for m in range(4):
            nc.tensor.matmul(s2p[:], psent_s[:, m:m + 1], u2_tiles[m][:],
                             start=(m == 0), stop=(m == 3))
        e2 = smpool.tile([1, B_PC], f32, tag="e2")
        nc.scalar.activation(e2[:], s2p[:], AF.Exp)
        sm2 = smpool.tile([1, BAGS_PC], f32, tag="sm2")
        nc.vector.tensor_reduce(
            sm2[:], e2[:].rearrange("p (g s) -> p g s", s=MS), AX.X, ALU.add)
        inv2 = smpool.tile([1, BAGS_PC], f32, tag="inv2")
        nc.vector.reciprocal(inv2[:], sm2[:])
        # broadcast e2/inv2 across partitions via DRAM bounce
        e2_d = dpool.tile([1, B_PC], f32, tag="e2_d")
        inv2_d = dpool.tile([1, BAGS_PC], f32, tag="inv2_d")
        nc.sync.dma_start(e2_d[:], e2[:])
        nc.sync.dma_start(inv2_d[:], inv2[:])
        e2r = acpool.tile([HT, B_PC], f32, tag="ach")
        nc.sync.dma_start(e2r[:], e2_d[0].partition_broadcast(HT))
        inv2r = ptpool.tile([HT, BAGS_PC], f32, tag="pt")
        nc.sync.dma_start(inv2r[:], inv2_d[0].partition_broadcast(HT))

        sv_tiles = []
        for m in range(4):
            tmp2 = tmppool.tile([HT, B_PC], bf16, tag="tmp")
            nc.vector.tensor_tensor(
                tmp2[:], wvbf[0:HT, m * B_PC:(m + 1) * B_PC], e2r[:],
                ALU.mult)
            sv = smpool.tile([HT, BAGS_PC], f32, tag=f"sv{m}")
            nc.vector.tensor_reduce(
                sv[:], tmp2[:].rearrange("p (g s) -> p g s", s=MS),
                AX.X, ALU.add)
            svb = smpool.tile([HT, BAGS_PC], bf16, tag=f"svb{m}")
            nc.vector.tensor_tensor(svb[:], sv[:], inv2r[:], ALU.mult)
            sv_tiles.append(svb)
        fcp = pp.tile([OUT, BAGS_PC], f32, tag="u")
        for m in range(4):
            nc.tensor.matmul(fcp[:], wfc_s[:, m * OUT:(m + 1) * OUT],
                             sv_tiles[m][:], start=(m == 0), stop=(m == 3))
        bos = smpool.tile([OUT, BAGS_PC], f32, tag="bos")
        if flags["HAS_BFC"]:
            nc.scalar.activation(bos[:], fcp[:], AF.Identity,
                                 bias=bias_s[0:OUT, 20:21])
        else:
            nc.scalar.copy(bos[:], fcp[:])
        nc.sync.dma_start(bo[:], bos[:])

    return nc


def _host_prep(inputs):
    """Build per-core input maps (weight images shared, x sharded)."""
    def as_np(a, dt=np.float32):
        return np.asarray(a, dtype=dt)

    bag = as_np(inputs["bag"])          # [512, 8, 64, 360]
    W_ih = [as_np(inputs["W_ih_f"]), as_np(inputs["W_ih_r"])]   # [690,360]
    W_hh = [as_np(inputs["W_hh_f"]), as_np(inputs["W_hh_r"])]   # [690,230]
    b_ih = [as_np(inputs["b_ih_f"]), as_np(inputs["b_ih_r"])]
    b_hh = [as_np(inputs["b_hh_f"]), as_np(inputs["b_hh_r"])]
    W_word = as_np(inputs["W_word"])    # [460,460]
    b_word = as_np(inputs["b_word"])
    proj_word = as_np(inputs["proj_word"])
    W_sent = as_np(inputs["W_sent"])
    b_sent = as_np(inputs["b_sent"])
    proj_sent = as_np(inputs["proj_sent"])
    fc_W = as_np(inputs["fc_W"])        # [53,460]
    fc_b = as_np(inputs["fc_b"])

    # weight images
    def padm(a):
        # pad last (M) dim from HT to 128 (full 128-col weight for FWL)
        pad = np.zeros(a.shape[:-1] + (128 - HT,), a.dtype)
        return np.concatenate([a, pad], axis=-1)

    def padk(a):
        # pad first (K=partition) dim to 128 rows
        pad = np.zeros((128 - a.shape[0],) + a.shape[1:], a.dtype)
        return np.concatenate([a, pad], axis=0)

    wih = np.stack(
        [W.reshape(6, HT, 3, KI).transpose(3, 2, 0, 1) for W in W_ih],
        axis=1)  # [KI, 2, 3, 6, HT]
    wih = padk(padm(np.ascontiguousarray(wih))).reshape(128, 2 * 3 * 6 * 128)
    whh = np.stack(
        [W.reshape(6, HT, 2, HT).transpose(3, 2, 0, 1) for W in W_hh],
        axis=1)  # [HT, 2, 2, 6, HT]
    whh = padk(padm(np.ascontiguousarray(whh))).reshape(128, 2 * 2 * 6 * 128)
    wword = padk(padm(np.ascontiguousarray(
        W_word.reshape(4, HT, 4, HT).transpose(1, 0, 2, 3)))).reshape(128, -1)
    wsent = padk(padm(np.ascontiguousarray(
        W_sent.reshape(4, HT, 4, HT).transpose(1, 0, 2, 3)))).reshape(128, -1)
    wfc = np.ascontiguousarray(
        fc_W.T.reshape(4, HT, OUT).transpose(1, 0, 2)).reshape(HT, -1)
    pword = np.ascontiguousarray(proj_word.reshape(4, HT).T)
    psent = np.ascontiguousarray(proj_sent.reshape(4, HT).T)

    bias = np.zeros((HT, 21), np.float32)
    for d in range(2):
        brz = (b_ih[d][:H2] + b_hh[d][:H2]).reshape(4, HT)
        bias[:, d * 4:(d + 1) * 4] = brz.T
        bias[:, 8 + d * 2:8 + (d + 1) * 2] = \
            b_ih[d][H2:].reshape(2, HT).T
    bias[:, 12:16] = b_word.reshape(4, HT).T
    bias[:, 16:20] = b_sent.reshape(4, HT).T
    bias[:OUT, 20] = fc_b
    bnhh = np.concatenate([b_hh[0][H2:], b_hh[1][H2:]])[None, :]

    flags = {
        "HAS_BRZ": bool(np.any(bias[:, 0:8])),
        "HAS_BNIH": bool(np.any(bias[:, 8:12])),
        "HAS_BNHH": bool(np.any(bnhh)),
        "HAS_BWORD": bool(np.any(bias[:, 12:16])),
        "HAS_BSENT": bool(np.any(bias[:, 16:20])),
        "HAS_BFC": bool(np.any(fc_b)),
    }

    shared = {
        "wih": wih.astype(BF16), "whh": whh.astype(BF16),
        "wword": wword.astype(BF16), "wsent": wsent.astype(BF16),
        "wfc": wfc.astype(BF16), "pword": pword.astype(BF16),
        "psent": psent.astype(BF16), "bias": bias,
        "bnhh": bnhh.astype(BF16),
    }

    in_maps = []
    for core in range(NCORES):
        bc = bag[core * BAGS_PC:(core + 1) * BAGS_PC]   # [64,8,64,360]
        x = bc.reshape(NSB, BS, T, IN).transpose(0, 2, 3, 1)  # [sb,t,f,b]
        x = np.ascontiguousarray(x).reshape(NSB, T, 3, KI, BS)
        xpad = np.zeros((NSB, T, 3, 128 - KI, BS), x.dtype)
        x = np.concatenate([x, xpad], axis=3)  # [sb,t,3,128,b]
        m = dict(shared)
        m["x_img"] = x.astype(BF16)
        in_maps.append(m)
    return in_maps, flags


def kernel(**inputs):
    in_maps, flags = _host_prep(inputs)

    key = tuple(sorted(flags.items()))
    if key not in _COMPILED:
        _COMPILED[key] = _build_program(flags)
    nc = _COMPILED[key]

    from concourse.bass_utils import run_bass_kernel_spmd
    res = run_bass_kernel_spmd(nc, in_maps, core_ids=list(range(NCORES)))

    pairs = np.asarray(inputs["pairs"], dtype=np.int64)  # [512,3]
    bag_out = np.concatenate(
        [res.results[c]["bo"].astype(np.float32).T for c in range(NCORES)],
        axis=0)  # [512, 53]
    out = np.zeros((DOCS, ENT, ENT, OUT), np.float32)
    out[pairs[:, 0], pairs[:, 1], pairs[:, 2]] = bag_out
    return out


# revision 20
# speedup vs baseline: 1.3290x; 1.2622x over previous
"""Trainium2 Bass kernel for nn_BiGRU_29901562314941.

Bag-of-sentences BiGRU + word/sentence attention + fc + scatter.

Strategy (8 NeuronCores, data-parallel over bags):
  - 512 bags -> 64 bags/core (= 512 sequences/core, T=64, IN=360).
  - Weights replicated; host pre-transposes everything into the exact
    SBUF images the device wants (feature-on-partitions, batch-on-free),
    in bf16, so the device does zero transposes/conversions.
  - Per core, 2 sub-batches of 256 sequences. Per sub-batch: forward GRU
    sweep, reverse GRU sweep (input projection fused into the recurrence
    PSUM accumulation - xg is never materialized in DRAM), then word
    attention (deferred softmax), accumulating word vectors.
  - Sentence attention + fc on device; host gathers per-core [53,64]
    outputs and scatters rows into the dense (32,8,8,53) result.
"""
import numpy as np
import ml_dtypes

# ---- problem constants (hardcoded per contract) ----
NB, MS, T, IN, H, OUT, DOCS, ENT = 512, 8, 64, 360, 230, 53, 32, 8
NCORES = 8
BAGS_PC = NB // NCORES          # 64 bags/core
B_PC = BAGS_PC * MS             # 512 seqs/core
NSB = 2                         # sub-batches per core
BS = B_PC // NSB                # 256 seqs per sub-batch
HT = 115                        # H = 2*HT
KI = 120                        # IN = 3*KI
H2 = 2 * H                      # 460
G3 = 3 * H                      # 690

BF16 = ml_dtypes.bfloat16

_COMPILED = {}


def _build_program(flags):
    nc = _build_program_nocompile(flags)
    nc.compile()
    return nc


def _build_program_nocompile(flags):
    """Build the per-core Bass/Tile program. flags: dict of
    HAS_BRZ / HAS_BNIH / HAS_BNHH / HAS_BWORD / HAS_BSENT / HAS_BFC."""
    from contextlib import ExitStack
    import concourse.bass as bass
    import concourse.tile as tile
    from concourse import bacc, mybir

    f32 = mybir.dt.float32
    bf16 = mybir.dt.bfloat16
    AF = mybir.ActivationFunctionType
    ALU = mybir.AluOpType
    AX = mybir.AxisListType

    nc = bacc.Bacc("TRN2", target_bir_lowering=False, debug=False,
                   num_devices=NCORES)

    # ---- DRAM I/O ----
    x_img = nc.dram_tensor("x_img", [NSB, T, 3, 128, BS], bf16,
                           kind="ExternalInput").ap()
    wih = nc.dram_tensor("wih", [128, 2 * 3 * 6 * 128], bf16,
                         kind="ExternalInput").ap()
    whh = nc.dram_tensor("whh", [128, 2 * 2 * 6 * 128], bf16,
                         kind="ExternalInput").ap()
    wword = nc.dram_tensor("wword", [128, 16 * 128], bf16,
                           kind="ExternalInput").ap()
    wsent = nc.dram_tensor("wsent", [128, 16 * 128], bf16,
                           kind="ExternalInput").ap()
    wfc = nc.dram_tensor("wfc", [HT, 4 * OUT], bf16,
                         kind="ExternalInput").ap()
    pword = nc.dram_tensor("pword", [HT, 4], bf16, kind="ExternalInput").ap()
    psent = nc.dram_tensor("psent", [HT, 4], bf16, kind="ExternalInput").ap()
    bias = nc.dram_tensor("bias", [HT, 21], f32, kind="ExternalInput").ap()
    bnhh = nc.dram_tensor("bnhh", [1, 2 * H], bf16, kind="ExternalInput").ap()
    bo = nc.dram_tensor("bo", [OUT, BAGS_PC], f32, kind="ExternalOutput").ap()

    with tile.TileContext(nc) as tc, ExitStack() as ctx:
        wpool = ctx.enter_context(tc.tile_pool(name="weights", bufs=1))
        xpool = ctx.enter_context(tc.tile_pool(name="x", bufs=2))
        xnpool = ctx.enter_context(tc.tile_pool(name="xn", bufs=2))
        gpool = ctx.enter_context(tc.tile_pool(name="gates", bufs=2))
        gpool1 = ctx.enter_context(tc.tile_pool(name="gates1", bufs=1))
        hpool = ctx.enter_context(tc.tile_pool(name="hstore", bufs=1))
        pp = ctx.enter_context(tc.tile_pool(name="ps", bufs=1, space="PSUM"))
        apool = ctx.enter_context(tc.tile_pool(name="attn", bufs=4))
        spool = ctx.enter_context(tc.tile_pool(name="sstage", bufs=2))
        acpool = ctx.enter_context(tc.tile_pool(name="achunk", bufs=2))
        tmppool = ctx.enter_context(tc.tile_pool(name="tmp", bufs=2))
        ptpool = ctx.enter_context(tc.tile_pool(name="partial", bufs=2))
        smpool = ctx.enter_context(tc.tile_pool(name="small", bufs=1))
        wvpool = ctx.enter_context(tc.tile_pool(name="wv", bufs=1))
        dpool = ctx.enter_context(
            tc.tile_pool(name="dram", bufs=1, space="DRAM"))

        # ---- load weights to SBUF ----
        def wtile(name, src, shape, dt):
            t = wpool.tile(shape, dt, tag=name)
            nc.sync.dma_start(t[:], src[:])
            return t

        wih_s = wtile("wih", wih, [128, 2 * 3 * 6 * 128], bf16)
        whh_s = wtile("whh", whh, [128, 2 * 2 * 6 * 128], bf16)
        wword_s = wtile("wword", wword, [128, 16 * 128], bf16)
        wsent_s = wtile("wsent", wsent, [128, 16 * 128], bf16)
        wfc_s = wtile("wfc", wfc, [HT, 4 * OUT], bf16)
        pword_s = wtile("pword", pword, [HT, 4], bf16)
        psent_s = wtile("psent", psent, [HT, 4], bf16)
        bias_s = wtile("bias", bias, [HT, 21], f32)
        bnhh_s = wtile("bnhh", bnhh, [1, 2 * H], bf16)
        ones_s = wpool.tile([1, BS], bf16, tag="ones")
        nc.vector.memset(ones_s[:], 1.0)

        # weight slice helpers
        def wih_sl(d, kc, m):
            c = ((d * 3 + kc) * 6 + m) * 128
            return wih_s[:, c:c + 128]

        def whh_sl(d, k2, m):
            c = ((d * 2 + k2) * 6 + m) * 128
            return whh_s[0:HT, c:c + 128]

        def wword_sl(kc, m):
            c = (kc * 4 + m) * 128
            return wword_s[0:HT, c:c + 128]

        def wsent_sl(kc, m):
            c = (kc * 4 + m) * 128
            return wsent_s[0:HT, c:c + 128]

        def bias_sl(col):
            return bias_s[:, col:col + 1]

        # persistent word-vector store: [115, (m,b)] cols m*512 + b
        wvbf = wvpool.tile([HT, 4 * B_PC], bf16, tag="wvbf")

        for sb in range(NSB):
            hf = hpool.tile([HT, 2 * T * BS], bf16, tag="hf")
            hr = hpool.tile([HT, 2 * T * BS], bf16, tag="hr")
            # word-attention accumulators (unnormalized exp weighting)
            acc = wvpool.tile([HT, 4 * BS], f32, tag="acc")
            dsum = smpool.tile([1, BS], f32, tag="dsum")

            # ---------------- GRU sweeps ----------------
            def sweep_step(d, hstore, step, rev):
                trow = (T - 1 - step) if rev else step
                tprev = (trow + 1) if rev else (trow - 1)
                first = step == 0

                xt = xpool.tile([128, 3 * BS], bf16, tag=f"x{d}")
                nc.sync.dma_start(
                    xt[:],
                    x_img[sb, trow].rearrange("kc f b -> f kc b"))

                def xsl(kc):
                    return xt[:, kc * BS:(kc + 1) * BS]

                def hsl(k2):
                    return hstore[:, k2 * T * BS + tprev * BS:
                                  k2 * T * BS + tprev * BS + BS]

                # xn psum: W_ih_n @ x_t -> SBUF bf16 (prefetchable)
                xnp = pp.tile([128, 2 * BS], f32, tag="xn")
                for m in (0, 1):
                    o = xnp[:, m * BS:(m + 1) * BS]
                    for kc in range(3):
                        nc.tensor.matmul(o, wih_sl(d, kc, 4 + m), xsl(kc),
                                         start=(kc == 0), stop=(kc == 2))
                xns = xnpool.tile([HT, 2 * BS], bf16, tag=f"xn{d}")
                if flags["HAS_BNIH"]:
                    for m in (0, 1):
                        nc.scalar.activation(
                            xns[:, m * BS:(m + 1) * BS],
                            xnp[0:HT, m * BS:(m + 1) * BS],
                            AF.Identity, bias=bias_sl(8 + d * 2 + m))
                else:
                    nc.scalar.copy(xns[:], xnp[0:HT, :])

                # r,z psum: W_ih_rz @ x + W_hh_rz @ h
                rzp = pp.tile([128, 4 * BS], f32, tag=f"rz{d}")
                for m in range(4):
                    o = rzp[:, m * BS:(m + 1) * BS]
                    for kc in range(3):
                        nc.tensor.matmul(o, wih_sl(d, kc, m), xsl(kc),
                                         start=(kc == 0),
                                         stop=(kc == 2 and first))
                    if not first:
                        for k2 in (0, 1):
                            nc.tensor.matmul(o, whh_sl(d, k2, m), hsl(k2),
                                             start=False, stop=(k2 == 1))
                rzs = gpool.tile([HT, 4 * BS], bf16, tag=f"rz{d}")
                if flags["HAS_BRZ"]:
                    for m in range(4):
                        nc.scalar.activation(
                            rzs[:, m * BS:(m + 1) * BS],
                            rzp[0:HT, m * BS:(m + 1) * BS],
                            AF.Sigmoid, bias=bias_sl(d * 4 + m))
                else:
                    nc.scalar.activation(rzs[:], rzp[0:HT, :], AF.Sigmoid)

                # n-gate hidden part
                need_np = (not first) or flags["HAS_BNHH"]
                if need_np:
                    np_ = pp.tile([128, 2 * BS], f32, tag="n")
                    for m in (0, 1):
                        o = np_[:, m * BS:(m + 1) * BS]
                        started = False
                        if flags["HAS_BNHH"]:
                            nc.tensor.matmul(
                                o[0:HT, :],
                                bnhh_s[:, d * H + m * HT:
                                       d * H + (m + 1) * HT],
                                ones_s[:], start=True, stop=first)
                            started = True
                        if not first:
                            for k2 in (0, 1):
                                nc.tensor.matmul(o, whh_sl(d, k2, 4 + m),
                                                 hsl(k2),
                                                 start=(not started and
                                                        k2 == 0),
                                                 stop=(k2 == 1))
                    tt = gpool1.tile([HT, 2 * BS], bf16, tag=f"t{d}")
                    nc.vector.tensor_tensor(tt[:], rzs[:, 0:2 * BS],
                                            np_[0:HT, :], ALU.mult)
                    nc.vector.tensor_tensor(tt[:], tt[:], xns[:], ALU.add)
                    tanh_in = tt[:]
                else:
                    tanh_in = xns[:]
                ns = gpool1.tile([HT, 2 * BS], bf16, tag=f"n{d}")
                nc.scalar.activation(ns[:], tanh_in, AF.Tanh)

                # h2 = n + z*(h_prev - n), written into hstore column t
                h3 = hstore[:].rearrange("p (m tb) -> p m tb", m=2)
                hview = h3[:, :, trow * BS:(trow + 1) * BS]
                ns3 = ns[:].rearrange("p (m b) -> p m b", m=2)
                z3 = rzs[:, 2 * BS:4 * BS].rearrange("p (m b) -> p m b", m=2)
                dd = gpool1.tile([HT, 2 * BS], bf16, tag=f"d{d}")
                dd3 = dd[:].rearrange("p (m b) -> p m b", m=2)
                if first:
                    nc.vector.tensor_scalar_mul(dd[:], ns[:], -1.0)
                else:
                    hprev3 = h3[:, :, tprev * BS:(tprev + 1) * BS]
                    nc.vector.tensor_tensor(dd3, hprev3, ns3, ALU.subtract)
                nc.vector.tensor_tensor(dd3, z3, dd3, ALU.mult)
                nc.vector.tensor_tensor(hview, ns3, dd3, ALU.add)

            # ---- word attention for one completed row t (unnormalized) ----
            def outchunk(kc, lo, n, p=HT):
                h_src = hf if kc < 2 else hr
                c = (kc % 2) * T * BS + lo
                return h_src[0:p, c:c + n]

            def attn_row(t, first_row):
                us_tiles = []
                for mp in (0, 1):
                    up = pp.tile([128, 2 * BS], f32, tag="u")
                    for mm in (0, 1):
                        m = mp * 2 + mm
                        o = up[:, mm * BS:(mm + 1) * BS]
                        for kc in range(4):
                            nc.tensor.matmul(o, wword_sl(kc, m),
                                             outchunk(kc, t * BS, BS),
                                             start=(kc == 0), stop=(kc == 3))
                    us = apool.tile([HT, 2 * BS], bf16, tag="us")
                    if flags["HAS_BWORD"]:
                        for mm in (0, 1):
                            nc.scalar.activation(
                                us[:, mm * BS:(mm + 1) * BS],
                                up[0:HT, mm * BS:(mm + 1) * BS], AF.Tanh,
                                bias=bias_sl(12 + mp * 2 + mm))
                    else:
                        nc.scalar.activation(us[:], up[0:HT, :], AF.Tanh)
                    us_tiles.append(us)
                sp = pp.tile([1, BS], f32, tag="u")
                for m in range(4):
                    nc.tensor.matmul(
                        sp[:], pword_s[:, m:m + 1],
                        us_tiles[m // 2][:, (m % 2) * BS:(m % 2 + 1) * BS],
                        start=(m == 0), stop=(m == 3))
                es = spool.tile([1, BS], bf16, tag="es")
                nc.scalar.activation(es[:], sp[:], AF.Exp)
                if first_row:
                    nc.vector.tensor_copy(dsum[:], es[:])
                else:
                    nc.vector.tensor_tensor(dsum[:], dsum[:], es[:], ALU.add)
                # broadcast exp weights across partitions via K=1 matmul
                ebp = pp.tile([128, BS], f32, tag="u")
                nc.tensor.matmul(ebp[:], ones_s[:, 0:128], es[:],
                                 start=True, stop=True)
                eb = apool.tile([HT, BS], bf16, tag="eb")
                nc.scalar.copy(eb[:], ebp[0:HT, :])
                for m in range(4):
                    accsl = acc[:, m * BS:(m + 1) * BS]
                    och = outchunk(m, t * BS, BS)
                    if first_row:
                        nc.vector.tensor_tensor(accsl, och, eb[:], ALU.mult)
                    else:
                        atmp = gpool1.tile([HT, BS], bf16, tag="atmp")
                        nc.vector.tensor_tensor(atmp[:], och, eb[:], ALU.mult)
                        nc.vector.tensor_tensor(accsl, accsl, atmp[:],
                                                ALU.add)

            for step in range(T):
                sweep_step(0, hf, step, rev=False)
                sweep_step(1, hr, step, rev=True)
                if step >= T // 2:
                    attn_row(step, first_row=(step == T // 2))
                    attn_row(T - 1 - step, first_row=False)

            # normalize: wv = acc / dsum  (broadcast 1/dsum via DRAM bounce)
            inv = smpool.tile([1, BS], f32, tag="inv")
            nc.vector.reciprocal(inv[:], dsum[:])
            inv_d = dpool.tile([1, BS], f32, tag="inv_d")
            nc.sync.dma_start(inv_d[:], inv[:])
            invr = ptpool.tile([HT, BS], f32, tag="pt")
            nc.sync.dma_start(invr[:], inv_d[0].partition_broadcast(HT))
            for m in range(4):
                nc.vector.tensor_tensor(
                    wvbf[:, m * B_PC + sb * BS:m * B_PC + sb * BS + BS],
                    acc[:, m * BS:(m + 1) * BS], invr[:], ALU.mult)

        # ---------------- sentence attention + fc ----------------
        u2_tiles = []
        for m in range(4):
            u2p = pp.tile([128, B_PC], f32, tag="u")
            for kc in range(4):
                nc.tensor.matmul(u2p[:], wsent_sl(kc, m),
                                 wvbf[:, kc * B_PC:(kc + 1) * B_PC],
                                 start=(kc == 0), stop=(kc == 3))
            u2s = apool.tile([HT, B_PC], bf16, tag="us")
            if flags["HAS_BSENT"]:
                nc.scalar.activation(u2s[:], u2p[0:HT, :], AF.Tanh,
                                     bias=bias_sl(16 + m))
            else:
                nc.scalar.activation(u2s[:], u2p[0:HT, :], AF.Tanh)
            u2_tiles.append(u2s)
        s2p = pp.tile([1, B_PC], f32, tag="u")
        for m in range(4):
            nc.tensor.matmul(s2p[:], psent_s[:, m:m + 1], u2_tiles[m][:],
                             start=(m == 0), stop=(m == 3))
        e2 = smpool.tile([1, B_PC], f32, tag="e2")
        nc.scalar.activation(e2[:], s2p[:], AF.Exp)
        sm2 = smpool.tile([1, BAGS_PC], f32, tag="sm2")
        nc.vector.tensor_reduce(
            sm2[:], e2[:].rearrange("p (g s) -> p g s", s=MS), AX.X, ALU.add)
        inv2 = smpool.tile([1, BAGS_PC], f32, tag="inv2")
        nc.vector.reciprocal(inv2[:], sm2[:])
        # broadcast e2/inv2 across partitions via DRAM bounce
        e2_d = dpool.tile([1, B_PC], f32, tag="e2_d")
        inv2_d = dpool.tile([1, BAGS_PC], f32, tag="inv2_d")
        nc.sync.dma_start(e2_d[:], e2[:])
        nc.sync.dma_start(inv2_d[:], inv2[:])
        e2r = acpool.tile([HT, B_PC], f32, tag="ach")
        nc.sync.dma_start(e2r[:], e2_d[0].partition_broadcast(HT))
        inv2r = ptpool.tile([HT, BAGS_PC], f32, tag="pt")
        nc.sync.dma_start(inv2r[:], inv2_d[0].partition_broadcast(HT))

        sv_tiles = []
        for m in range(4):
            tmp2 = tmppool.tile([HT, B_PC], bf16, tag="tmp")
            nc.vector.tensor_tensor(
                tmp2[:], wvbf[:, m * B_PC:(m + 1) * B_PC], e2r[:],
                ALU.mult)
            sv = smpool.tile([HT, BAGS_PC], f32, tag=f"sv{m}")
            nc.vector.tensor_reduce(
                sv[:], tmp2[:].rearrange("p (g s) -> p g s", s=MS),
                AX.X, ALU.add)
            svb = smpool.tile([HT, BAGS_PC], bf16, tag=f"svb{m}")
            nc.vector.tensor_tensor(svb[:], sv[:], inv2r[:], ALU.mult)
            sv_tiles.append(svb)
        fcp = pp.tile([OUT, BAGS_PC], f32, tag="u")
        for m in range(4):
            nc.tensor.matmul(fcp[:], wfc_s[:, m * OUT:(m + 1) * OUT],
                             sv_tiles[m][:], start=(m == 0), stop=(m == 3))
        bos = smpool.tile([OUT, BAGS_PC], f32, tag="bos")
        if flags["HAS_BFC"]:
            nc.scalar.activation(bos[:], fcp[:], AF.Identity,
                                 bias=bias_s[0:OUT, 20:21])
        else:
            nc.scalar.copy(bos[:], fcp[:])
        nc.sync.dma_start(bo[:], bos[:])

    return nc


def _host_prep(inputs):
    """Build per-core input maps (weight images shared, x sharded)."""
    def as_np(a, dt=np.float32):
        return np.asarray(a, dtype=dt)

    bag = as_np(inputs["bag"])          # [512, 8, 64, 360]
    W_ih = [as_np(inputs["W_ih_f"]), as_np(inputs["W_ih_r"])]   # [690,360]
    W_hh = [as_np(inputs["W_hh_f"]), as_np(inputs["W_hh_r"])]   # [690,230]
    b_ih = [as_np(inputs["b_ih_f"]), as_np(inputs["b_ih_r"])]
    b_hh = [as_np(inputs["b_hh_f"]), as_np(inputs["b_hh_r"])]
    W_word = as_np(inputs["W_word"])    # [460,460]
    b_word = as_np(inputs["b_word"])
    proj_word = as_np(inputs["proj_word"])
    W_sent = as_np(inputs["W_sent"])
    b_sent = as_np(inputs["b_sent"])
    proj_sent = as_np(inputs["proj_sent"])
    fc_W = as_np(inputs["fc_W"])        # [53,460]
    fc_b = as_np(inputs["fc_b"])

    # weight images
    def padm(a):
        # pad last (M) dim from HT to 128 (full 128-col weight for FWL)
        pad = np.zeros(a.shape[:-1] + (128 - HT,), a.dtype)
        return np.concatenate([a, pad], axis=-1)

    def padk(a):
        # pad first (K=partition) dim to 128 rows
        pad = np.zeros((128 - a.shape[0],) + a.shape[1:], a.dtype)
        return np.concatenate([a, pad], axis=0)

    wih = np.stack(
        [W.reshape(6, HT, 3, KI).transpose(3, 2, 0, 1) for W in W_ih],
        axis=1)  # [KI, 2, 3, 6, HT]
    wih = padk(padm(np.ascontiguousarray(wih))).reshape(128, 2 * 3 * 6 * 128)
    whh = np.stack(
        [W.reshape(6, HT, 2, HT).transpose(3, 2, 0, 1) for W in W_hh],
        axis=1)  # [HT, 2, 2, 6, HT]
    whh = padk(padm(np.ascontiguousarray(whh))).reshape(128, 2 * 2 * 6 * 128)
    wword = padk(padm(np.ascontiguousarray(
        W_word.reshape(4, HT, 4, HT).transpose(1, 0, 2, 3)))).reshape(128, -1)
    wsent = padk(padm(np.ascontiguousarray(
        W_sent.reshape(4, HT, 4, HT).transpose(1, 0, 2, 3)))).reshape(128, -1)
    wfc = np.ascontiguousarray(
        fc_W.T.reshape(4, HT, OUT).transpose(1, 0, 2)).reshape(HT, -1)
    pword = np.ascontiguousarray(proj_word.reshape(4, HT).T)
    psent = np.ascontiguousarray(proj_sent.reshape(4, HT).T)

    bias = np.zeros((HT, 21), np.float32)
    for d in range(2):
        brz = (b_ih[d][:H2] + b_hh[d][:H2]).reshape(4, HT)
        bias[:, d * 4:(d + 1) * 4] = brz.T
        bias[:, 8 + d * 2:8 + (d + 1) * 2] = \
            b_ih[d][H2:].reshape(2, HT).T
    bias[:, 12:16] = b_word.reshape(4, HT).T
    bias[:, 16:20] = b_sent.reshape(4, HT).T
    bias[:OUT, 20] = fc_b
    bnhh = np.concatenate([b_hh[0][H2:], b_hh[1][H2:]])[None, :]

    flags = {
        "HAS_BRZ": bool(np.any(bias[:, 0:8])),
        "HAS_BNIH": bool(np.any(bias[:, 8:12])),
        "HAS_BNHH": bool(np.any(bnhh)),
        "HAS_BWORD": bool(np.any(bias[:, 12:16])),
        "HAS_BSENT": bool(np.any(bias[:, 16:20])),
        "HAS_BFC": bool(np.any(fc_b)),
    }

    shared = {
        "wih": wih.astype(BF16), "whh": whh.astype(BF16),
        "wword": wword.astype(BF16), "wsent": wsent.astype(BF16),
        "wfc": wfc.astype(BF16), "pword": pword.astype(BF16),
        "psent": psent.astype(BF16), "bias": bias,
        "bnhh": bnhh.astype(BF16),
    }

    in_maps = []
    for core in range(NCORES):
        bc = bag[core * BAGS_PC:(core + 1) * BAGS_PC]   # [64,8,64,360]
        x = bc.reshape(NSB, BS, T, IN).transpose(0, 2, 3, 1)  # [sb,t,f,b]
        x = np.ascontiguousarray(x).reshape(NSB, T, 3, KI, BS)
        xpad = np.zeros((NSB, T, 3, 128 - KI, BS), x.dtype)
        x = np.concatenate([x, xpad], axis=3)  # [sb,t,3,128,b]
        m = dict(shared)
        m["x_img"] = x.astype(BF16)
        in_maps.append(m)
    return in_maps, flags


def kernel(**inputs):
    in_maps, flags = _host_prep(inputs)

    key = tuple(sorted(flags.items()))
    if key not in _COMPILED:
        _COMPILED[key] = _build_program(flags)
    nc = _COMPILED[key]

    from concourse.bass_utils import run_bass_kernel_spmd
    res = run_bass_kernel_spmd(nc, in_maps, core_ids=list(range(NCORES)))

    pairs = np.asarray(inputs["pairs"], dtype=np.int64)  # [512,3]
    bag_out = np.concatenate(
        [res.results[c]["bo"].astype(np.float32).T for c in range(NCORES)],
        axis=0)  # [512, 53]
    out = np.zeros((DOCS, ENT, ENT, OUT), np.float32)
    out[pairs[:, 0], pairs[:, 1], pairs[:, 2]] = bag_out
    return out
